# revision 20
# baseline (speedup 1.0000x reference)
"""LocallyConnected2dV2 Trainium2 kernel (bf16, raw pre-context input stream).

Math: out[b, a, bp] = sum_{k,l} xpad[b, 5a+k, 5bp+l] * kw[a, bp, k, l] + bias[a, bp]

Strategy (8 cores, data-parallel over batch, 128 images/core):
  - Host: transpose each core's x shard to [col j', row r, batch b]; cast
    bf16; compact W into per-image-row banded blocks Wh[r, j', 50] (bf16).
  - Input stream: ALL 8 combined x+w chunk DMAs plus the aux (bias/ones)
    DMA are raw-bass instructions issued BEFORE the TileContext: they
    bypass the ~1.2us context-entry prologue and drain strictly FIFO on
    the two HWDGE rings (sync: even chunks; scalar: aux + odd chunks) —
    no Tile DMA scheduling, no issue-lane stalls.
  - PE gating: NX dispatches in order, so a tiny constant dummy matmul
    carrying a semaphore wait (attached post-scheduling — the Tile sim
    cannot model the external DMAs) gates every later Ldweights/Matmult.
    One gate is emitted right before the first consumer of each chunk.
  - Compute: per output-row group g (5 x 125 psum cols), accumulate
    ~30 row matmuls (bf16, fp32 psum); bias enters last via a K=1
    ones-vector matmul; DVE casts psum->bf16; per-group output DMA.
"""

import numpy as np
import ml_dtypes

BF16 = ml_dtypes.bfloat16

B = 1024
R = 128           # image rows = cols
NCORES = 8
BS = B // NCORES  # 128 batch per core
NK = 625
WP = 132
NG = 5            # output-row groups (5 a's each)
GW = 125          # psum cols per group
# chunk row spans: 6x16 + 4x8; the 8-row tail chunks let each DMA's
# completion receipt (~2us HBM round-trip) overlap the next one's transfer
CHUNK_OFF = [0, 16, 32, 48, 64, 80, 96, 104, 112, 120]
CHUNK_LEN = [16, 16, 16, 16, 16, 16, 8, 8, 8, 8]
NCH = len(CHUNK_OFF)


def _chunk_of_row(r):
    for ic in range(NCH):
        if CHUNK_OFF[ic] <= r < CHUNK_OFF[ic] + CHUNK_LEN[ic]:
            return ic
    raise ValueError(r)


def _a0_of_row(r):
    return min(max((r - 3) // 5, 0), 23)


def _group_rows(g):
    return range(max(0, 25 * g - 2), min(R - 1, 25 * g + 27) + 1)


def _row_parts(r, g):
    """Matmul pieces row r contributes to group g."""
    a0 = _a0_of_row(r)
    lo = 5 * g
    if a0 >= lo and a0 + 1 < lo + 5:
        return [((a0 - lo) * 25, 50, 0)]
    parts = []
    for ai, a in ((0, a0), (1, a0 + 1)):
        if lo <= a < lo + 5:
            parts.append(((a - lo) * 25, 25, ai * 25))
    return parts


def prep_weights(W, bias):
    """W [17424, 625], bias [25,25] -> wt [128, 128*50], bs [1, 625]."""
    W = np.asarray(W, np.float32)
    i = np.arange(NK)
    si = (i // 25) * 5
    sj = (i % 25) * 5
    rows = ((si[:, None, None] + np.arange(10)[None, :, None]) * WP
            + sj[:, None, None] + np.arange(10)[None, None, :])
    kw = W[rows.reshape(NK, 100), i[:, None]].reshape(25, 25, 10, 10)

    r = np.arange(R)[:, None, None]
    jp = np.arange(R)[None, :, None]
    c = np.arange(50)[None, None, :]
    ai = c // 25
    bp = c % 25
    a = np.clip((r - 3) // 5, 0, 23) + ai
    k = r + 2 - 5 * a
    l = jp + 2 - 5 * bp
    valid = (k >= 0) & (k < 10) & (l >= 0) & (l < 10)
    Wh = np.where(valid, kw[a, bp, np.clip(k, 0, 9), np.clip(l, 0, 9)], 0.0)
    Wh = Wh.astype(np.float32)                       # [r, j', 50]
    wt = np.ascontiguousarray(Wh.transpose(1, 0, 2)).reshape(R, R * 50)
    bs = np.ascontiguousarray(np.asarray(bias, np.float32).reshape(1, NK))
    return wt, bs


def _build_nc():
    import concourse.bass as bass
    import concourse.mybir as mybir
    import concourse.tile as tile
    from concourse import bacc

    bf16 = mybir.dt.bfloat16
    nc = bacc.Bacc("TRN2", target_bir_lowering=False, debug=False)
    xw_cols = sum(n * (BS + 50) for n in CHUNK_LEN)
    xw = nc.dram_tensor("xw", [R, xw_cols], bf16, kind="ExternalInput").ap()
    aux = nc.dram_tensor("aux", [1, NK + BS], bf16, kind="ExternalInput").ap()
    out = nc.dram_tensor("out", [NG * BS, GW], bf16, kind="ExternalOutput").ap()

    # raw pre-context input stream
    es = [nc.alloc_sbuf_tensor(f"e{i}", [R, CHUNK_LEN[i] * (BS + 50)], bf16)
          for i in range(NCH)]
    aux_sb = nc.alloc_sbuf_tensor("auxsb", [1, NK + BS], bf16)
    sems = [nc.alloc_semaphore(f"esem{i}") for i in range(NCH)]
    saux = nc.alloc_semaphore("sauxsem")
    # ring order — sync: ch0, aux, ch2, ch4, ch6; scalar: ch1, ch3, ch5, ch7.
    # aux rides the sync ring behind ch0 (delivered ~12us, first needed ~18us)
    # so the scalar ring starts streaming ch1 with no issue delay ahead of it.
    xw_off = [0]
    for n in CHUNK_LEN:
        xw_off.append(xw_off[-1] + n * (BS + 50))

    def chunk_dma(i):
        eng = nc.sync if i % 2 == 0 else nc.scalar
        eng.dma_start(es[i].ap(),
                      xw[:, xw_off[i]:xw_off[i + 1]]).then_inc(sems[i], 16)
    chunk_dma(0)
    chunk_dma(1)
    nc.sync.dma_start(aux_sb.ap(), aux[:]).then_inc(saux, 16)
    for i in range(2, NCH):
        chunk_dma(i)

    bias_t = aux_sb.ap()[:, 0:NK]
    ones_t = aux_sb.ap()[:, NK:NK + BS]

    gates = []   # (dummy matmul inst, semaphore) — waits attached post-sched

    with tile.TileContext(nc) as tc:
        with (
            tc.tile_pool(name="ps", bufs=5, space=bass.MemorySpace.PSUM) as ps_pool,
            tc.tile_pool(name="dps", bufs=1, space=bass.MemorySpace.PSUM) as dps_pool,
            tc.tile_pool(name="ob", bufs=1) as ob_pool,
        ):
            one_bf = nc.const_aps.aps[(mybir.dt.bfloat16, 1.0)]
            dps = dps_pool.tile([1, 1], mybir.dt.float32, tag="dummy")

            def gate(sem):
                inst = nc.tensor.matmul(
                    dps[0:1, 0:1], one_bf[0:1, 0:1], one_bf[0:1, 0:1],
                    start=True, stop=True, skip_group_check=True)
                gates.append((inst, sem))

            seen = set()
            out_sb = ob_pool.tile([BS, NK], bf16, tag="osb")
            for g in range(NG):
                ps = ps_pool.tile([BS, GW], mybir.dt.float32, tag="ps")
                started = False
                for r in _group_rows(g):
                    ic = _chunk_of_row(r)
                    if ic not in seen:
                        gate(sems[ic])
                        seen.add(ic)
                    rr = r - CHUNK_OFF[ic]
                    ct = es[ic].ap()
                    lhsT = ct[:, rr * BS:(rr + 1) * BS]
                    wb = CHUNK_LEN[ic] * BS + rr * 50
                    for (pc, n, wc) in _row_parts(r, g):
                        nc.tensor.matmul(ps[:, pc:pc + n], lhsT,
                                         ct[:, wb + wc:wb + wc + n],
                                         start=not started, stop=False)
                        started = True
                # bias enters last (stop matmul of the group)
                if "aux" not in seen:
                    gate(saux)
                    seen.add("aux")
                nc.tensor.matmul(ps[:, 0:GW], ones_t,
                                 bias_t[:, g * GW:(g + 1) * GW],
                                 start=False, stop=True)
                nc.vector.tensor_copy(
                    out_sb[:, g * GW:(g + 1) * GW], ps[:])
                nc.scalar.dma_start(out[g * BS:(g + 1) * BS, :],
                                    out_sb[:, g * GW:(g + 1) * GW])

    # Attach the stream waits post-scheduling: the Tile simulator does not
    # model the pre-context DMAs and would report a false deadlock.
    for inst, sem in gates:
        inst._wait_ge(sem, 16)
    nc.compile()
    return nc


_NC_CACHE = []


def _get_nc():
    if not _NC_CACHE:
        _NC_CACHE.append(_build_nc())
    return _NC_CACHE[0]


def make_in_maps(x, W, bias):
    x = np.asarray(x, np.float32)
    wt, bsv = prep_weights(W, bias)
    wt16 = wt.astype(BF16)
    auxv = np.concatenate(
        [bsv.astype(BF16), np.ones((1, BS), BF16)], axis=1)
    in_maps = []
    for c in range(NCORES):
        xc = x[c * BS:(c + 1) * BS]                      # [b, r, j']
        xtv = np.ascontiguousarray(
            xc.transpose(2, 1, 0)).astype(BF16).reshape(R, R * BS)
        parts = []
        for ic in range(NCH):
            o, n = CHUNK_OFF[ic], CHUNK_LEN[ic]
            parts.append(xtv[:, o * BS:(o + n) * BS])
            parts.append(wt16[:, o * 50:(o + n) * 50])
        xwv = np.ascontiguousarray(np.concatenate(parts, axis=1))
        in_maps.append({"xw": xwv, "aux": auxv})
    return in_maps


def run(x, W, bias, trace=False, **kw):
    from concourse import bass_utils
    nc = _get_nc()
    res = bass_utils.run_bass_kernel_spmd(
        nc, make_in_maps(x, W, bias), list(range(NCORES)), trace=trace, **kw)
    outs = []
    for c in range(NCORES):
        o = np.asarray(res.results[c]["out"])            # [NG*BS, GW] bf16
        o = o.reshape(NG, BS, GW).transpose(1, 0, 2)     # [BS, NG, GW]
        outs.append(o.reshape(BS, 25, 25).astype(np.float32))
    return np.concatenate(outs, axis=0), res


def kernel(**inputs):
    out, _ = run(inputs["x"], inputs["W"], inputs["bias"])
    return out


# revision 22
# speedup vs baseline: 1.0927x; 1.0927x over previous
"""LocallyConnected2dV2 Trainium2 kernel (bf16, raw pre-context input stream).

Math: out[b, a, bp] = sum_{k,l} xpad[b, 5a+k, 5bp+l] * kw[a, bp, k, l] + bias[a, bp]

Strategy (8 cores, data-parallel over batch, 128 images/core):
  - Host: transpose each core's x shard to [col j', row r, batch b]; cast
    bf16; compact W into per-image-row banded blocks Wh[r, j', 50] (bf16).
  - Input stream: ALL 8 combined x+w chunk DMAs plus the aux (bias/ones)
    DMA are raw-bass instructions issued BEFORE the TileContext: they
    bypass the ~1.2us context-entry prologue and drain strictly FIFO on
    the two HWDGE rings (sync: even chunks; scalar: aux + odd chunks) —
    no Tile DMA scheduling, no issue-lane stalls.
  - PE gating: NX dispatches in order, so a tiny constant dummy matmul
    carrying a semaphore wait (attached post-scheduling — the Tile sim
    cannot model the external DMAs) gates every later Ldweights/Matmult.
    One gate is emitted right before the first consumer of each chunk.
  - Compute: per output-row group g (5 x 125 psum cols), accumulate
    ~30 row matmuls (bf16, fp32 psum); bias enters last via a K=1
    ones-vector matmul; DVE casts psum->bf16; per-group output DMA.
"""

import numpy as np
import ml_dtypes

BF16 = ml_dtypes.bfloat16

B = 1024
R = 128           # image rows = cols
NCORES = 8
BS = B // NCORES  # 128 batch per core
NK = 625
WP = 132
NG = 5            # output-row groups (5 a's each)
GW = 125          # psum cols per group
# chunk row spans: 6x16 + 4x8; the 8-row tail chunks let each DMA's
# completion receipt (~2us HBM round-trip) overlap the next one's transfer
CHUNK_OFF = [0, 16, 32, 48, 64, 80, 96, 104, 112, 120]
CHUNK_LEN = [16, 16, 16, 16, 16, 16, 8, 8, 8, 8]
NCH = len(CHUNK_OFF)


def _chunk_of_row(r):
    for ic in range(NCH):
        if CHUNK_OFF[ic] <= r < CHUNK_OFF[ic] + CHUNK_LEN[ic]:
            return ic
    raise ValueError(r)


def _a0_of_row(r):
    return min(max((r - 3) // 5, 0), 23)


def _group_rows(g):
    return range(max(0, 25 * g - 2), min(R - 1, 25 * g + 27) + 1)


def _row_parts(r, g):
    """Matmul pieces row r contributes to group g."""
    a0 = _a0_of_row(r)
    lo = 5 * g
    if a0 >= lo and a0 + 1 < lo + 5:
        return [((a0 - lo) * 25, 50, 0)]
    parts = []
    for ai, a in ((0, a0), (1, a0 + 1)):
        if lo <= a < lo + 5:
            parts.append(((a - lo) * 25, 25, ai * 25))
    return parts


def prep_weights(W, bias):
    """W [17424, 625], bias [25,25] -> wt [128, 128*50], bs [1, 625]."""
    W = np.asarray(W, np.float32)
    i = np.arange(NK)
    si = (i // 25) * 5
    sj = (i % 25) * 5
    rows = ((si[:, None, None] + np.arange(10)[None, :, None]) * WP
            + sj[:, None, None] + np.arange(10)[None, None, :])
    kw = W[rows.reshape(NK, 100), i[:, None]].reshape(25, 25, 10, 10)

    r = np.arange(R)[:, None, None]
    jp = np.arange(R)[None, :, None]
    c = np.arange(50)[None, None, :]
    ai = c // 25
    bp = c % 25
    a = np.clip((r - 3) // 5, 0, 23) + ai
    k = r + 2 - 5 * a
    l = jp + 2 - 5 * bp
    valid = (k >= 0) & (k < 10) & (l >= 0) & (l < 10)
    Wh = np.where(valid, kw[a, bp, np.clip(k, 0, 9), np.clip(l, 0, 9)], 0.0)
    Wh = Wh.astype(np.float32)                       # [r, j', 50]
    wt = np.ascontiguousarray(Wh.transpose(1, 0, 2)).reshape(R, R * 50)
    bs = np.ascontiguousarray(np.asarray(bias, np.float32).reshape(1, NK))
    return wt, bs


def _build_nc():
    import concourse.bass as bass
    import concourse.mybir as mybir
    import concourse.tile as tile
    from concourse import bacc

    bf16 = mybir.dt.bfloat16
    nc = bacc.Bacc("TRN2", target_bir_lowering=False, debug=False)
    xw_cols = sum(n * (BS + 50) for n in CHUNK_LEN)
    xw = nc.dram_tensor("xw", [R, xw_cols], bf16, kind="ExternalInput").ap()
    aux = nc.dram_tensor("aux", [1, NK + BS], bf16, kind="ExternalInput").ap()
    out = nc.dram_tensor("out", [NG * BS, GW], bf16, kind="ExternalOutput").ap()

    # raw pre-context input stream
    es = [nc.alloc_sbuf_tensor(f"e{i}", [R, CHUNK_LEN[i] * (BS + 50)], bf16)
          for i in range(NCH)]
    aux_sb = nc.alloc_sbuf_tensor("auxsb", [1, NK + BS], bf16)
    sems = [nc.alloc_semaphore(f"esem{i}") for i in range(NCH)]
    saux = nc.alloc_semaphore("sauxsem")
    # ring order — sync: ch0, aux, ch2, ch4, ch6; scalar: ch1, ch3, ch5, ch7.
    # aux rides the sync ring behind ch0 (delivered ~12us, first needed ~18us)
    # so the scalar ring starts streaming ch1 with no issue delay ahead of it.
    xw_off = [0]
    for n in CHUNK_LEN:
        xw_off.append(xw_off[-1] + n * (BS + 50))

    def chunk_dma(i):
        eng = nc.sync if i % 2 == 0 else nc.scalar
        eng.dma_start(es[i].ap(),
                      xw[:, xw_off[i]:xw_off[i + 1]]).then_inc(sems[i], 16)
    chunk_dma(0)
    chunk_dma(1)
    nc.sync.dma_start(aux_sb.ap(), aux[:]).then_inc(saux, 16)
    for i in range(2, NCH):
        chunk_dma(i)

    bias_t = aux_sb.ap()[:, 0:NK]
    ones_t = aux_sb.ap()[:, NK:NK + BS]

    gates = []   # (dummy matmul inst, semaphore) — waits attached post-sched

    with tile.TileContext(nc) as tc:
        with (
            tc.tile_pool(name="ps", bufs=4, space=bass.MemorySpace.PSUM) as ps_pool,
            tc.tile_pool(name="dps", bufs=1, space=bass.MemorySpace.PSUM) as dps_pool,
            tc.tile_pool(name="ob", bufs=1) as ob_pool,
        ):
            one_bf = nc.const_aps.aps[(mybir.dt.bfloat16, 1.0)]
            dps = dps_pool.tile([1, 1], mybir.dt.float32, tag="dummy")

            def gate(sem):
                inst = nc.tensor.matmul(
                    dps[0:1, 0:1], one_bf[0:1, 0:1], one_bf[0:1, 0:1],
                    start=True, stop=True, skip_group_check=True)
                gates.append((inst, sem))

            seen = set()
            out_sb = ob_pool.tile([BS, NK], bf16, tag="osb")

            def emit_group(g, splits):
                """splits: list of (lo, hi) column ranges, each its own
                psum tile so earlier ranges cast+DMA while later rows of
                the group still accumulate."""
                tiles = []
                for si, (lo, hi) in enumerate(splits):
                    tag = "ps" if len(splits) == 1 else f"ps{g}_{si}"
                    pool = ps_pool if len(splits) == 1 else dps_pool
                    ptile = pool.tile([BS, hi - lo], mybir.dt.float32,
                                      tag=tag)
                    tiles.append([ptile, lo, hi, False])
                rows = list(_group_rows(g))
                # dry pass: last row contributing to each split tile
                last_writer = [-1] * len(splits)
                for r in rows:
                    for (pc, n, wc) in _row_parts(r, g):
                        for ti, (lo, hi) in enumerate(splits):
                            if max(pc, lo) < min(pc + n, hi):
                                last_writer[ti] = r

                def finish_tile(ti):
                    if "aux" not in seen:
                        gate(saux)
                        seen.add("aux")
                    t = tiles[ti]
                    lo, hi = t[1], t[2]
                    nc.tensor.matmul(
                        t[0][:, 0:hi - lo], ones_t,
                        bias_t[:, g * GW + lo:g * GW + hi],
                        start=False, stop=True)
                    nc.vector.tensor_copy(
                        out_sb[:, g * GW + lo:g * GW + hi],
                        t[0][:, 0:hi - lo])
                    nc.scalar.dma_start(
                        out[g * BS:(g + 1) * BS, lo:hi],
                        out_sb[:, g * GW + lo:g * GW + hi])

                for r in rows:
                    ic = _chunk_of_row(r)
                    if ic not in seen:
                        gate(sems[ic])
                        seen.add(ic)
                    rr = r - CHUNK_OFF[ic]
                    ct = es[ic].ap()
                    lhsT = ct[:, rr * BS:(rr + 1) * BS]
                    wb = CHUNK_LEN[ic] * BS + rr * 50
                    for (pc, n, wc) in _row_parts(r, g):
                        for ti, t in enumerate(tiles):
                            lo, hi = t[1], t[2]
                            a0c, b0c = max(pc, lo), min(pc + n, hi)
                            if a0c >= b0c:
                                continue
                            nc.tensor.matmul(
                                t[0][:, a0c - lo:b0c - lo], lhsT,
                                ct[:, wb + wc + (a0c - pc):
                                    wb + wc + (b0c - pc)],
                                start=not t[3], stop=False)
                            t[3] = True
                    # finish any tile whose last contributing row was r:
                    # its bias/cast/out chain then overlaps later rows
                    for ti in range(len(splits)):
                        if last_writer[ti] == r:
                            finish_tile(ti)

            for g in range(NG - 1):
                emit_group(g, [(0, GW)])
            emit_group(NG - 1, [(0, 75), (75, GW)])

    # Attach the stream waits post-scheduling: the Tile simulator does not
    # model the pre-context DMAs and would report a false deadlock.
    for inst, sem in gates:
        inst._wait_ge(sem, 16)
    nc.compile()
    return nc


_NC_CACHE = []


def _get_nc():
    if not _NC_CACHE:
        _NC_CACHE.append(_build_nc())
    return _NC_CACHE[0]


def make_in_maps(x, W, bias):
    x = np.asarray(x, np.float32)
    wt, bsv = prep_weights(W, bias)
    wt16 = wt.astype(BF16)
    auxv = np.concatenate(
        [bsv.astype(BF16), np.ones((1, BS), BF16)], axis=1)
    in_maps = []
    for c in range(NCORES):
        xc = x[c * BS:(c + 1) * BS]                      # [b, r, j']
        xtv = np.ascontiguousarray(
            xc.transpose(2, 1, 0)).astype(BF16).reshape(R, R * BS)
        parts = []
        for ic in range(NCH):
            o, n = CHUNK_OFF[ic], CHUNK_LEN[ic]
            parts.append(xtv[:, o * BS:(o + n) * BS])
            parts.append(wt16[:, o * 50:(o + n) * 50])
        xwv = np.ascontiguousarray(np.concatenate(parts, axis=1))
        in_maps.append({"xw": xwv, "aux": auxv})
    return in_maps


def run(x, W, bias, trace=False, **kw):
    from concourse import bass_utils
    nc = _get_nc()
    res = bass_utils.run_bass_kernel_spmd(
        nc, make_in_maps(x, W, bias), list(range(NCORES)), trace=trace, **kw)
    outs = []
    for c in range(NCORES):
        o = np.asarray(res.results[c]["out"])            # [NG*BS, GW] bf16
        o = o.reshape(NG, BS, GW).transpose(1, 0, 2)     # [BS, NG, GW]
        outs.append(o.reshape(BS, 25, 25).astype(np.float32))
    return np.concatenate(outs, axis=0), res


def kernel(**inputs):
    out, _ = run(inputs["x"], inputs["W"], inputs["bias"])
    return out
